# revision 10
# baseline (speedup 1.0000x reference)
"""VQ codebook encoding (soft-assignment aggregation) on 8 Trainium2 NeuronCores.

Reference computation (per batch b, with Xf = X[b] reshaped to [N, D]):
    dist[n,k] = ||x_n||^2 - 2<x_n, c_k> + ||c_k||^2
    A = softmax_k(scale_k * dist[n,k])
    E[k,d] = sum_n A[n,k] * Xf[n,d] - (sum_n A[n,k]) * C[k,d]

Sharding: data-parallel over B (8 batches -> 8 cores), no collectives.

Per-core dataflow (X[b] arrives d-major as [D=512, N=16384]):
  - For each tile of 128 n-values: load the four [128d x 128n] slices of X as
    PE weights once each; from the same weights issue (a) a matmul against a
    128x128 identity -> transposed tile Xf[n,d] in PSUM (needed because the
    output aggregation contracts over n, which must sit on partitions), and
    (b) a matmul against the pre-scaled codebook (-2*scale_k*C^T) -> the
    cross-term of the scaled distance, accumulated over the 4 d-chunks.
  - ScalarE copies Xf PSUM->SBUF (stream operand of the second matmul) and a
    Square-activation with accum_out produces ||x_n||^2 per partition.
  - VectorE assembles logits scale_k*(x2 - 2xc) and adds the scale_k*||c_k||^2
    bias; ScalarE exponentiates with accum_out producing the softmax
    denominator for free; reciprocal + tensor_scalar normalize.
  - PE accumulates E[k,d] (A as weights, Xf as stream) and S[k]=sum_n A[n,k]
    into persistent PSUM banks across all 128 n-tiles; the epilogue computes
    E - S*C and DMAs out [32, 512].
"""

import numpy as np

import concourse.bass as bass
import concourse.tile as tile
from concourse import bacc, mybir
from concourse.bass_utils import run_bass_kernel_spmd

F32 = mybir.dt.float32
AF = mybir.ActivationFunctionType
ALU = mybir.AluOpType

B, D, K, N = 8, 512, 32, 16384
P = 128                 # partitions
DC = D // P             # 4 d-chunks
NT = N // P             # 128 n-tiles per core
SG_N = 2048             # n-values per DMA super-group (1 MiB per d-chunk slice)
NSG = N // SG_N         # 8 super-groups
TPG = SG_N // P         # 16 n-tiles per super-group


def _build_bass():
    nc = bacc.Bacc(None, target_bir_lowering=False)

    x_d = nc.declare_dram_parameter("x", [D, N], F32, isOutput=False)
    ctm2s_d = nc.declare_dram_parameter("ctm2s", [D, K], F32, isOutput=False)
    scaleb_d = nc.declare_dram_parameter("scaleb", [P, K], F32, isOutput=False)
    bb_d = nc.declare_dram_parameter("bb", [P, K], F32, isOutput=False)
    ident_d = nc.declare_dram_parameter("ident", [P, P], F32, isOutput=False)
    ones_d = nc.declare_dram_parameter("onescol", [P, 1], F32, isOutput=False)
    cs_d = nc.declare_dram_parameter("cs", [K, D], F32, isOutput=False)
    e_d = nc.declare_dram_parameter("e", [K, D], F32, isOutput=True)

    with tile.TileContext(nc) as tc:
        with (
            tc.tile_pool(name="consts", bufs=1) as cpool,
            tc.tile_pool(name="xin", bufs=2 * DC) as xin_pool,
            tc.tile_pool(name="xf_sb", bufs=3) as xf_pool,
            tc.tile_pool(name="smalls", bufs=4) as sm_pool,
            tc.tile_pool(name="scratch", bufs=1) as scr_pool,
            tc.tile_pool(name="xf_ps", bufs=2, space="PSUM") as xfps_pool,
            tc.tile_pool(name="sl_ps", bufs=2, space="PSUM") as slps_pool,
            tc.tile_pool(name="acc_ps", bufs=1, space="PSUM") as accps_pool,
        ):
            # ---- constants to SBUF ----
            ctm2s = cpool.tile([P, DC, K], F32)   # chunk c at [:, c, :]
            nc.sync.dma_start(
                ctm2s[:], ctm2s_d.rearrange("(c p) k -> p c k", p=P)
            )
            scaleb = cpool.tile([P, K], F32)
            nc.sync.dma_start(scaleb[:], scaleb_d[:])
            bb = cpool.tile([P, K], F32)
            nc.sync.dma_start(bb[:], bb_d[:])
            ident = cpool.tile([P, P], F32)
            nc.sync.dma_start(ident[:], ident_d[:])
            onescol = cpool.tile([P, 1], F32)
            nc.sync.dma_start(onescol[:], ones_d[:])
            cs = cpool.tile([K, D], F32)
            nc.sync.dma_start(cs[:], cs_d[:])

            sq_scr = scr_pool.tile([P, DC * P], F32)  # dead store for Square

            e_ps = accps_pool.tile([K, D], F32)
            s_ps = accps_pool.tile([K, 1], F32)

            for sg in range(NSG):
                xin = []
                for c in range(DC):
                    t = xin_pool.tile([P, SG_N], F32, tag="xin")
                    nc.sync.dma_start(
                        t[:], x_d[c * P:(c + 1) * P, sg * SG_N:(sg + 1) * SG_N]
                    )
                    xin.append(t)

                for ti in range(TPG):
                    nt = sg * TPG + ti
                    xf_ps = xfps_pool.tile([P, DC * P], F32)
                    sl_ps = slps_pool.tile([P, K], F32)
                    for c in range(DC):
                        lhsT = xin[c][:, ti * P:(ti + 1) * P]
                        # transpose: Xf tile [n, d-chunk] (plain matmul vs
                        # identity so the weight load is shared with the
                        # cross-term matmul below)
                        nc.tensor.matmul(
                            xf_ps[:, c * P:(c + 1) * P], lhsT, ident[:],
                            start=True, stop=True,
                        )
                        # cross-term: -2*scale_k*<x_n, c_k>, accumulated
                        nc.tensor.matmul(
                            sl_ps[:], lhsT, ctm2s[:, c, :],
                            start=(c == 0), stop=(c == DC - 1),
                        )

                    # Xf PSUM -> SBUF (stream operand for the E matmul)
                    xf_sb = xf_pool.tile([P, DC * P], F32)
                    nc.scalar.copy(xf_sb[:], xf_ps[:])

                    # x2[n] = sum_d Xf[n,d]^2 via Square + accumulate
                    x2 = sm_pool.tile([P, 1], F32, tag="x2")
                    nc.scalar.activation(
                        sq_scr[:], xf_ps[:], AF.Square, accum_out=x2[:]
                    )

                    # logits: scale_k * x2[n] + (-2 scale_k xc)
                    sl_sb = sm_pool.tile([P, K], F32, tag="sl")
                    nc.vector.scalar_tensor_tensor(
                        sl_sb[:], scaleb[:], x2[:], sl_ps[:],
                        op0=ALU.mult, op1=ALU.add,
                    )

                    # + scale_k*c2_k bias -> full scaled distance (<= 0)
                    sl2 = sm_pool.tile([P, K], F32, tag="sl2")
                    nc.vector.tensor_add(sl2[:], sl_sb[:], bb[:])

                    # Q = exp(logits); denom = sum_k Q via accum_out
                    q = sm_pool.tile([P, K], F32, tag="q")
                    denom = sm_pool.tile([P, 1], F32, tag="den")
                    nc.scalar.activation(q[:], sl2[:], AF.Exp, accum_out=denom[:])

                    rcol = sm_pool.tile([P, 1], F32, tag="rc")
                    nc.vector.reciprocal(rcol[:], denom[:])

                    a = sm_pool.tile([P, K], F32, tag="a")
                    nc.vector.tensor_scalar_mul(a[:], q[:], rcol[:])

                    # E[k,d] += A.T @ Xf ; S[k] += A.T @ 1
                    nc.tensor.matmul(
                        e_ps[:], a[:], xf_sb[:],
                        start=(nt == 0), stop=(nt == NT - 1),
                        skip_group_check=True,
                    )
                    nc.tensor.matmul(
                        s_ps[:], a[:], onescol[:],
                        start=(nt == 0), stop=(nt == NT - 1),
                        skip_group_check=True,
                    )

            # epilogue: E = e_ps - S*C
            s_neg = sm_pool.tile([K, 1], F32, tag="sn")
            nc.scalar.activation(s_neg[:], s_ps[:], AF.Copy, scale=-1.0)
            e_sb = xf_pool.tile([K, D], F32, tag="eout")
            nc.vector.scalar_tensor_tensor(
                e_sb[:], cs[:], s_neg[:], e_ps[:],
                op0=ALU.mult, op1=ALU.add,
            )
            nc.sync.dma_start(e_d[:], e_sb[:])

    nc.compile()
    return nc


_CACHED = {}


def _get_nc():
    if "nc" not in _CACHED:
        _CACHED["nc"] = _build_bass()
    return _CACHED["nc"]


def kernel(X, codewords, scale, _trace=False):
    X = np.asarray(X, dtype=np.float32)
    codewords = np.asarray(codewords, dtype=np.float32)
    scale = np.asarray(scale, dtype=np.float32)

    Xr = np.ascontiguousarray(X.reshape(B, D, N))

    c2 = (codewords.astype(np.float64) ** 2).sum(axis=1)
    ctm2s = np.ascontiguousarray(
        (-2.0 * scale[None, :] * codewords.T).astype(np.float32)
    )
    scaleb = np.broadcast_to(scale[None, :], (P, K)).copy()
    bb = np.broadcast_to(
        (scale.astype(np.float64) * c2).astype(np.float32)[None, :], (P, K)
    ).copy()
    ident = np.eye(P, dtype=np.float32)
    onescol = np.ones((P, 1), dtype=np.float32)
    cs = np.ascontiguousarray(codewords)

    consts = dict(
        ctm2s=ctm2s, scaleb=scaleb, bb=bb,
        ident=ident, onescol=onescol, cs=cs,
    )
    in_maps = [dict(x=np.ascontiguousarray(Xr[b]), **consts) for b in range(B)]

    nc = _get_nc()
    res = run_bass_kernel_spmd(nc, in_maps, list(range(B)), trace=_trace)
    out = np.stack([res.results[b]["e"] for b in range(B)]).astype(np.float32)
    if _trace:
        kernel.last_results = res
    return out
